# revision 1
# baseline (speedup 1.0000x reference)
"""Trainium2 Bass kernel for nn_BTGRule (BTG rule scoring over a span chart).

Reference computation:
    L = span_rep @ Wl + bl            # [65, 65, 512]
    R = span_rep @ Wr + br            # [65, 65, 512]
    H = tanh(L[i, j] + R[j, k])       # over valid triples i < j < k
    scores[i, j, k] = H @ Wout + bout # [65, 65, 65, 2], zeros at invalid triples

Strategy (8 NeuronCores, SPMD — one program, per-core data):
  * Only valid (i, j) pairs (i < j <= 63) are computed; k runs over (j, 64].
  * Pairs are grouped by j into 8 groups g = ceil(j/8); group g has 8 j-slots
    (padded) and n = g pairs per core per j-slot, so every core runs an
    identical instruction stream. The i assignment per core is pure input data.
  * Host pre-transposes span_rep columns so every matmul has its contraction
    dim on partitions; the device never transposes.
  * On device per core: R^T projection (PE -> PSUM), fused broadcast add
    L + R via one DVE tensor_tensor per (group, h-tile) reading PSUM,
    tanh (ACT), score matmul vs Wout (PE), bias-add copy (ACT), DMA out.
"""

import numpy as np

N1 = 65          # chart side (N + 1)
HID = 512        # hidden size
OUT = 2          # output size
NCORES = 8
HT = HID // 128  # 4 h-tiles

# ---------------------------------------------------------------------------
# Pair-group layout (all compile-time constants, identical on host and device)
# ---------------------------------------------------------------------------
# group g (1..8): j in [8(g-1)+1, min(8g, 63)], padded to 8 j-slots.
# W[g] = max k-width in group = 64 - (8(g-1)+1) + ... = 63 - 8(g-1)
# n[g] = g pairs per core per j-slot.


def _build_layout():
    groups = []
    r_off = 0   # column offset into the R / span_cols buffer (8*W per group)
    q_off = 0   # column offset into the Lsel / span_sel buffer (8*n per group)
    s_off = 0   # column offset into the S / output buffer (n*8*W per group)
    for g in range(1, 9):
        js = [8 * (g - 1) + 1 + t for t in range(8)]
        js = [j if j <= 63 else None for j in js]
        W = 63 - 8 * (g - 1)
        n = g
        groups.append(dict(g=g, js=js, W=W, n=n, r_off=r_off, q_off=q_off,
                           s_off=s_off))
        r_off += 8 * W
        q_off += 8 * n
        s_off += n * 8 * W
    return groups, r_off, q_off, s_off


GROUPS, RCOLS, QCOLS, SCOLS = _build_layout()  # 2240, 288, 7392

_COMPILED = None


def _build_program():
    """Trace + compile the single SPMD program. Returns (nc, meta)."""
    import concourse.bacc as bacc
    import concourse.mybir as mybir
    import concourse.tile as tile

    f32 = mybir.dt.float32
    nc = bacc.Bacc("TRN2", target_bir_lowering=False, debug=False,
                   num_devices=NCORES)

    span_cols_d = nc.declare_dram_parameter("span_cols", [HID, RCOLS], f32,
                                            isOutput=False)
    span_sel_d = nc.declare_dram_parameter("span_sel", [HID, QCOLS], f32,
                                           isOutput=False)
    wl_d = nc.declare_dram_parameter("wl", [HID, HID], f32, isOutput=False)
    wr_d = nc.declare_dram_parameter("wr", [HID, HID], f32, isOutput=False)
    woutp_d = nc.declare_dram_parameter("woutp", [128, OUT * HT], f32,
                                        isOutput=False)
    blbr_d = nc.declare_dram_parameter("blbr", [128, HT], f32, isOutput=False)
    bout_d = nc.declare_dram_parameter("bout", [OUT, 1], f32, isOutput=False)
    out_d = nc.declare_dram_parameter("out", [OUT, SCOLS], f32, isOutput=True)

    ident = mybir.ActivationFunctionType.Identity
    tanh = mybir.ActivationFunctionType.Tanh

    with tile.TileContext(nc) as tc:
        with (
            tc.tile_pool(name="const", bufs=1) as cpool,
            tc.tile_pool(name="spool", bufs=12) as spool,
            tc.tile_pool(name="ps_r", bufs=4, space="PSUM") as ps_r,
            tc.tile_pool(name="ps_sc", bufs=2, space="PSUM") as ps_sc,
        ):
            # ---- resident inputs -------------------------------------------
            span_t = []
            sel_t = []
            wl_t = []
            wr_t = []
            for t in range(HT):
                st = cpool.tile([128, RCOLS], f32, tag=f"span{t}")
                nc.sync.dma_start(st[:], span_cols_d[t * 128:(t + 1) * 128, :])
                span_t.append(st)
                se = cpool.tile([128, QCOLS], f32, tag=f"sel{t}")
                nc.sync.dma_start(se[:], span_sel_d[t * 128:(t + 1) * 128, :])
                sel_t.append(se)
                w1 = cpool.tile([128, HID], f32, tag=f"wl{t}")
                nc.sync.dma_start(w1[:], wl_d[t * 128:(t + 1) * 128, :])
                wl_t.append(w1)
                w2 = cpool.tile([128, HID], f32, tag=f"wr{t}")
                nc.sync.dma_start(w2[:], wr_d[t * 128:(t + 1) * 128, :])
                wr_t.append(w2)
            woutp_t = cpool.tile([128, OUT * HT], f32, tag="woutp")
            nc.sync.dma_start(woutp_t[:], woutp_d[:])
            blbr_t = cpool.tile([128, HT], f32, tag="blbr")
            nc.sync.dma_start(blbr_t[:], blbr_d[:])
            bout_t = cpool.tile([OUT, 1], f32, tag="bout")
            nc.sync.dma_start(bout_t[:], bout_d[:])
            out_sb = cpool.tile([OUT, SCOLS], f32, tag="out")

            # ---- Lsel^T[h_out, q] = Wl.T @ span_sel^T (+ bl + br) ----------
            lsel_t = []
            for to in range(HT):
                pl = ps_r.tile([128, QCOLS], f32, tag="psr")
                for ti in range(HT):
                    nc.tensor.matmul(pl[:], wl_t[ti][:, to * 128:(to + 1) * 128],
                                     sel_t[ti][:], start=(ti == 0),
                                     stop=(ti == HT - 1))
                lt = cpool.tile([128, QCOLS], f32, tag=f"lsel{to}")
                nc.scalar.activation(lt[:], pl[:], ident,
                                     bias=blbr_t[:, to:to + 1])
                lsel_t.append(lt)

            # ---- per group: R proj -> fused add -> tanh -> score matmul ----
            for grp in GROUPS:
                W, n = grp["W"], grp["n"]
                r0 = grp["r_off"]
                q0 = grp["q_off"]
                s0 = grp["s_off"]
                cols = n * 8 * W
                s_tiles = []
                for to in range(HT):
                    pr = ps_r.tile([128, 8 * W], f32, tag="psr")
                    for ti in range(HT):
                        nc.tensor.matmul(
                            pr[:], wr_t[ti][:, to * 128:(to + 1) * 128],
                            span_t[ti][:, r0:r0 + 8 * W],
                            start=(ti == 0), stop=(ti == HT - 1))
                    s = spool.tile([128, cols], f32, tag="s")
                    out_v = s[:].rearrange("p (a jj w) -> p a jj w", a=n, jj=8)
                    in0 = (pr[:].rearrange("p (jj w) -> p jj w", jj=8)
                           .unsqueeze(1).broadcast_to([128, n, 8, W]))
                    in1 = (lsel_t[to][:, q0:q0 + 8 * n]
                           .rearrange("p (a jj) -> p a jj", a=n)
                           .unsqueeze(3).broadcast_to([128, n, 8, W]))
                    nc.vector.tensor_tensor(out_v, in0, in1,
                                            op=mybir.AluOpType.add)
                    nc.scalar.activation(s[:], s[:], tanh)
                    s_tiles.append(s)

                # scores: psum [2, chunk<=1024], matmul sub-chunks <= 512
                c = 0
                while c < cols:
                    cw = min(1024, cols - c)
                    psc = ps_sc.tile([OUT, cw], f32, tag="pssc")
                    cc = 0
                    while cc < cw:
                        ccw = min(512, cw - cc)
                        for to in range(HT):
                            nc.tensor.matmul(
                                psc[:, cc:cc + ccw],
                                woutp_t[:, OUT * to:OUT * (to + 1)],
                                s_tiles[to][:, c + cc:c + cc + ccw],
                                start=(to == 0), stop=(to == HT - 1))
                        cc += ccw
                    nc.scalar.activation(out_sb[:, s0 + c:s0 + c + cw], psc[:],
                                         ident, bias=bout_t[:])
                    c += cw

            nc.sync.dma_start(out_d[:], out_sb[:])

    nc.compile()
    return nc


def _get_compiled():
    global _COMPILED
    if _COMPILED is None:
        _COMPILED = _build_program()
    return _COMPILED


# ---------------------------------------------------------------------------
# Host-side sharding / unsharding
# ---------------------------------------------------------------------------

def make_inputs(span_rep, Wl, bl, Wr, br, Wout, bout):
    """Build the per-core input maps."""
    span_rep = np.ascontiguousarray(np.asarray(span_rep, dtype=np.float32))
    Wl = np.ascontiguousarray(np.asarray(Wl, dtype=np.float32))
    Wr = np.ascontiguousarray(np.asarray(Wr, dtype=np.float32))
    Wout = np.asarray(Wout, dtype=np.float32)
    bl = np.asarray(bl, dtype=np.float32)
    br = np.asarray(br, dtype=np.float32)
    bout = np.asarray(bout, dtype=np.float32)

    # span_cols: replicated. Columns are span_rep[j, k]^T for each group's
    # j-slots, k in (j, j+W], zero-padded past k=64.
    span_cols = np.zeros((HID, RCOLS), dtype=np.float32)
    for grp in GROUPS:
        W = grp["W"]
        for jj, j in enumerate(grp["js"]):
            if j is None:
                continue
            w = 64 - j
            c0 = grp["r_off"] + jj * W
            span_cols[:, c0:c0 + w] = span_rep[j, j + 1:65, :].T

    # span_sel: per core. Column q = a*8 + jj of group g holds
    # span_rep[i, j]^T with i = a*8 + core (if that is a valid pair).
    sels = []
    for core in range(NCORES):
        sel = np.zeros((HID, QCOLS), dtype=np.float32)
        for grp in GROUPS:
            n = grp["n"]
            for jj, j in enumerate(grp["js"]):
                if j is None:
                    continue
                for a in range(n):
                    i = a * 8 + core
                    if i < j:
                        sel[:, grp["q_off"] + a * 8 + jj] = span_rep[i, j, :]
        sels.append(sel)

    woutp = np.ascontiguousarray(
        Wout.reshape(HT, 128, OUT).transpose(1, 0, 2).reshape(128, HT * OUT))
    blbr = np.ascontiguousarray((bl + br).reshape(HT, 128).T)
    bout2 = np.ascontiguousarray(bout.reshape(OUT, 1))

    in_maps = []
    for core in range(NCORES):
        in_maps.append({
            "span_cols": span_cols,
            "span_sel": sels[core],
            "wl": Wl,
            "wr": Wr,
            "woutp": woutp,
            "blbr": blbr,
            "bout": bout2,
        })
    return in_maps


def scatter_outputs(core_outs):
    """Assemble the full [65, 65, 65, 2] output from per-core [2, SCOLS]."""
    full = np.zeros((N1, N1, N1, OUT), dtype=np.float32)
    for core in range(NCORES):
        oc = core_outs[core]
        for grp in GROUPS:
            W, n = grp["W"], grp["n"]
            for jj, j in enumerate(grp["js"]):
                if j is None:
                    continue
                w = 64 - j
                for a in range(n):
                    i = a * 8 + core
                    if i < j:
                        c0 = grp["s_off"] + (a * 8 + jj) * W
                        full[i, j, j + 1:65, :] = oc[:, c0:c0 + w].T
    return full


def kernel(span_rep, Wl, bl, Wr, br, Wout, bout):
    from concourse.bass_utils import run_bass_kernel_spmd

    nc = _get_compiled()
    in_maps = make_inputs(span_rep, Wl, bl, Wr, br, Wout, bout)
    res = run_bass_kernel_spmd(nc, in_maps, core_ids=list(range(NCORES)))
    core_outs = [res.results[c]["out"] for c in range(NCORES)]
    return scatter_outputs(core_outs)


if __name__ == "__main__":
    rng = np.random.default_rng(0)
    s = 1.0 / np.sqrt(HID)
    inputs = dict(
        span_rep=rng.standard_normal((N1, N1, HID)).astype(np.float32),
        Wl=(rng.standard_normal((HID, HID)) * s).astype(np.float32),
        bl=np.zeros(HID, np.float32),
        Wr=(rng.standard_normal((HID, HID)) * s).astype(np.float32),
        br=np.zeros(HID, np.float32),
        Wout=(rng.standard_normal((HID, OUT)) * s).astype(np.float32),
        bout=np.zeros(OUT, np.float32),
    )
    out = kernel(**inputs)
    print("out", out.shape, out.dtype, np.abs(out).max())


# revision 21
# speedup vs baseline: 16121.7821x; 16121.7821x over previous
"""Trainium2 Bass kernel for nn_BTGRule (BTG rule scoring over a span chart).

Reference computation:
    L = span_rep @ Wl + bl            # [65, 65, 512]
    R = span_rep @ Wr + br            # [65, 65, 512]
    H = tanh(L[i, j] + R[j, k])       # over valid triples i < j < k
    scores[i, j, k] = H @ Wout + bout # [65, 65, 65, 2], zeros at invalid triples

Strategy (8 NeuronCores, SPMD — one program, per-core data):
  * Only valid (i, j) pairs (i < j <= 63) are computed; k runs over (j, 64].
  * Pairs are grouped by j into 8 groups g = ceil(j/8); group g has 8 j-slots
    (padded) and n = g pairs per core per j-slot, so every core runs an
    identical instruction stream. The i assignment per core is pure input data.
  * Host pre-transposes span_rep columns so every matmul has its contraction
    dim on partitions; the device never transposes. Inputs are packed so each
    R-projection chunk needs exactly one DMA (compute starts immediately).
  * On device per core: R^T projection (PE -> PSUM, float32r full-rate),
    fused broadcast add L + R via one DVE tensor_tensor per (group, h-tile)
    reading PSUM directly, tanh (ACT), score matmul vs Wout (PE), bias-add
    copy (DVE), DMA out. Two-half software pipeline keeps the PE bubble-free.
"""

import numpy as np

N1 = 65          # chart side (N + 1)
HID = 512        # hidden size
OUT = 2          # output size
NCORES = 8
HT = HID // 128  # 4 h-tiles

# ---------------------------------------------------------------------------
# Pair-group layout (all compile-time constants, identical on host and device)
# ---------------------------------------------------------------------------
# group g (1..8): j in [8(g-1)+1, min(8g, 63)], padded to 8 j-slots.
# W[g] = max k-width in group = 63 - 8(g-1);  n[g] = g pairs per core per slot.


def _build_layout():
    groups = []
    r_off = 0   # column offset into the R / span_cols space (8*W per group)
    q_off = 0   # column offset into the Lsel / span_sel space (8*n per group)
    s_off = 0   # column offset into the S / output space (n*8*W per group)
    for g in range(1, 9):
        js = [8 * (g - 1) + 1 + t for t in range(8)]
        js = [j if j <= 63 else None for j in js]
        W = 63 - 8 * (g - 1)
        n = g
        groups.append(dict(g=g, js=js, W=W, n=n, r_off=r_off, q_off=q_off,
                           s_off=s_off))
        r_off += 8 * W
        q_off += 8 * n
        s_off += n * 8 * W
    return groups, r_off, q_off, s_off


GROUPS, RCOLS, QCOLS, SCOLS = _build_layout()  # 2240, 288, 7392

# R-projection chunks: merged so every float32r matmul has >=256 output cols
# (below 256 it runs 4 cycles/row). The last chunk is zero-padded to 256.
RCHUNK_IDXS = [[0], [1], [2], [3], [4, 5], [6, 7]]


def _build_rchunks():
    chunks = []
    off4 = 0
    for idxs in RCHUNK_IDXS:
        grps = [GROUPS[gi] for gi in idxs]
        rbase = grps[0]["r_off"]
        rcols = max(sum(8 * g["W"] for g in grps), 256)
        assert rcols <= 512
        chunks.append(dict(idxs=idxs, rbase=rbase, rcols=rcols, off4=off4))
        off4 += HT * rcols
    return chunks, off4


RCHUNKS, SPANP_COLS = _build_rchunks()   # packed span cols = 4 * sum(rcols)

_COMPILED = None


def _build_program(reps=1):
    """Trace + compile the single SPMD program. reps>1 wraps the body in an
    on-device repeat loop (benchmarking only)."""
    import contextlib

    import concourse.bacc as bacc
    import concourse.mybir as mybir
    import concourse.tile as tile

    f32 = mybir.dt.float32
    f16 = mybir.dt.float16
    nc = bacc.Bacc("TRN2", target_bir_lowering=False, debug=False,
                   num_devices=NCORES)

    spanp_d = nc.declare_dram_parameter("spanp", [128, SPANP_COLS], f16,
                                        isOutput=False)
    selp_d = nc.declare_dram_parameter("selp", [128, HT * QCOLS], f16,
                                       isOutput=False)
    wp_d = nc.declare_dram_parameter("wp", [128, 2 * HT * HID], f16,
                                     isOutput=False)
    misc_d = nc.declare_dram_parameter("misc", [128, 16], f32, isOutput=False)
    out_d = nc.declare_dram_parameter("out", [OUT, SCOLS], f32, isOutput=True)

    ident = mybir.ActivationFunctionType.Identity
    tanh = mybir.ActivationFunctionType.Tanh

    # float32r: same 4-byte fp32 data, but the PE runs the matmul at full
    # rate (fp32 proper costs 4 cycles/row on TRN2).
    def r32(ap):
        return ap.bitcast(mybir.dt.float32r)

    def even_chunks(total, cap=512):
        # near-equal pieces, multiples of 8 (fp32r matmul ISA restriction
        # disallows odd output widths), each within one PSUM bank
        k = -(-total // cap)
        base = -(-total // (k * 8)) * 8
        return [base] * (k - 1) + [total - base * (k - 1)]

    with tile.TileContext(nc) as tc:
        with (
            tc.tile_pool(name="const", bufs=1) as cpool,
            tc.tile_pool(name="ps_r", bufs=5, space="PSUM") as ps_r,
            tc.tile_pool(name="ps_sc", bufs=3, space="PSUM") as ps_sc,
            tc.For_i(0, reps, 1, hint_engines=(mybir.EngineType.PE,
                                               mybir.EngineType.DVE,
                                               mybir.EngineType.Activation,
                                               mybir.EngineType.SP))
            if reps > 1 else contextlib.nullcontext(),
        ):
            # ---- input DMAs + interleaved warm-up --------------------------
            # Weights are packed as [128, (to*HT+ti)*128] blocks and DMA'd
            # per h_out so the Lsel and first R-projection matmuls can start
            # as soon as their own slices land. DMA issue order is chosen to
            # minimize the time until the first DVE broadcast-add.
            misc_t = cpool.tile([128, 16], f32, tag="misc")
            nc.sync.dma_start(r32(misc_t[:]), r32(misc_d[:]))
            blbr_t = misc_t[:, 0:HT]
            bout_t = misc_t[0:OUT, HT:HT + 1]
            woutp_t = misc_t[:, HT + 1:HT + 1 + OUT * HT]
            sel_t = cpool.tile([128, HT * QCOLS], f16, tag="sel")
            nc.sync.dma_start(sel_t[:], selp_d[:])
            w_t = cpool.tile([128, 2 * HT * HID], f16, tag="w")
            span_c = [None] * len(RCHUNKS)

            def dma_w(to):  # one DMA brings both Wl and Wr blocks for h_out=to
                nc.sync.dma_start(
                    w_t[:, to * 2 * HID:(to + 1) * 2 * HID],
                    wp_d[:, to * 2 * HID:(to + 1) * 2 * HID])

            def dma_span(ci):
                ch = RCHUNKS[ci]
                st = cpool.tile([128, HT * ch["rcols"]], f16, tag=f"spanc{ci}")
                nc.sync.dma_start(
                    st[:],
                    spanp_d[:, ch["off4"]:ch["off4"] + HT * ch["rcols"]])
                span_c[ci] = st

            dma_w(0)
            dma_span(0)
            for to in range(1, HT):
                dma_w(to)
            for ci in range(1, len(RCHUNKS)):
                dma_span(ci)
            out_sb = cpool.tile([OUT, SCOLS], f32, tag="out")

            def wblk(kind, to, ti):  # kind 0 = Wl, 1 = Wr
                c0 = to * 2 * HID + kind * HID + ti * 128
                return w_t[:, c0:c0 + 128]

            # ---- Lsel(to) interleaved with R-chunk-0(to) -------------------
            lsel_t = []
            ch0 = RCHUNKS[0]
            pr0_tiles = []
            for to in range(HT):
                pl = ps_r.tile([128, QCOLS], f32, tag="psr")
                for ti in range(HT):
                    nc.tensor.matmul(
                        pl[:], wblk(0, to, ti),
                        sel_t[:, ti * QCOLS:(ti + 1) * QCOLS],
                        start=(ti == 0), stop=(ti == HT - 1))
                lt = cpool.tile([128, QCOLS], f32, tag=f"lsel{to}")
                nc.scalar.activation(lt[:], pl[:], ident,
                                     bias=blbr_t[:, to:to + 1])
                lsel_t.append(lt)
                pr = ps_r.tile([128, ch0["rcols"]], f32, tag="psr")
                for ti in range(HT):
                    nc.tensor.matmul(
                        pr[:], wblk(1, to, ti),
                        span_c[0][:, ti * ch0["rcols"]:
                                  (ti + 1) * ch0["rcols"]],
                        start=(ti == 0), stop=(ti == HT - 1))
                pr0_tiles.append(pr)

            # ---- two-half software pipeline --------------------------------
            # Per half: phase A = R projection (PE) -> fused broadcast add
            # (DVE, reads PSUM) -> tanh (ACT); phase B = score matmuls (PE)
            # -> bias-add copy (DVE). By the time the PE reaches a score
            # matmul its tanh finished while the PE ran other projections.
            for half in (RCHUNKS[:4], RCHUNKS[4:]):
                s_tiles = {}
                pos = 0
                for ci, ch in enumerate(half):
                    rcols = ch["rcols"]
                    sc_t = span_c[RCHUNKS.index(ch)]
                    if ch is RCHUNKS[0]:
                        pr_tiles = pr0_tiles
                    else:
                        pr_tiles = []
                        for to in range(HT):
                            pr = ps_r.tile([128, rcols], f32, tag="psr")
                            for ti in range(HT):
                                nc.tensor.matmul(
                                    pr[:], wblk(1, to, ti),
                                    sc_t[:, ti * rcols:(ti + 1) * rcols],
                                    start=(ti == 0), stop=(ti == HT - 1))
                            pr_tiles.append(pr)

                    for gi in ch["idxs"]:
                        grp = GROUPS[gi]
                        W, n = grp["W"], grp["n"]
                        q0 = grp["q_off"]
                        loc0 = grp["r_off"] - ch["rbase"]
                        cols = n * 8 * W
                        s = cpool.tile([128, HT * cols], f32, tag=f"s{pos}")
                        pos += 1
                        s_tiles[grp["g"]] = s
                        for to in range(HT):
                            out_v = (s[:, to * cols:(to + 1) * cols]
                                     .rearrange("p (a jj w) -> p a jj w",
                                                a=n, jj=8))
                            in0 = (pr_tiles[to][:, loc0:loc0 + 8 * W]
                                   .rearrange("p (jj w) -> p jj w", jj=8)
                                   .unsqueeze(1).broadcast_to([128, n, 8, W]))
                            in1 = (lsel_t[to][:, q0:q0 + 8 * n]
                                   .rearrange("p (a jj) -> p a jj", a=n)
                                   .unsqueeze(3).broadcast_to([128, n, 8, W]))
                            nc.vector.tensor_tensor(
                                out_v.bitcast(mybir.dt.float32r), in0, in1,
                                op=mybir.AluOpType.add)
                        # one tanh over all four h-tile sections
                        nc.scalar.activation(r32(s[:]), r32(s[:]), tanh)

                # phase B: score matmuls + bias-add copies for this half
                for ch in half:
                    for gi in ch["idxs"]:
                        grp = GROUPS[gi]
                        W, n = grp["W"], grp["n"]
                        s0 = grp["s_off"]
                        cols = n * 8 * W
                        s = s_tiles[grp["g"]]
                        c = 0
                        for ci2, ccw in enumerate(even_chunks(cols)):
                            psc = ps_sc.tile([OUT, ccw], f32, tag="pssc")
                            for to in range(HT):
                                nc.tensor.matmul(
                                    psc[:],
                                    r32(woutp_t[:, OUT * to:OUT * (to + 1)]),
                                    r32(s[:, to * cols + c:
                                          to * cols + c + ccw]),
                                    start=(to == 0), stop=(to == HT - 1))
                            if (grp["g"] + ci2) % 2 == 0:
                                nc.vector.tensor_scalar_add(
                                    out_sb[:, s0 + c:s0 + c + ccw], psc[:],
                                    bout_t)
                            else:
                                nc.scalar.activation(
                                    out_sb[:, s0 + c:s0 + c + ccw], psc[:],
                                    ident, bias=bout_t)
                            c += ccw

            nc.sync.dma_start(out_d[:], out_sb[:])

    nc.compile()
    return nc


def _get_compiled():
    global _COMPILED
    if _COMPILED is None:
        _COMPILED = _build_program()
    return _COMPILED


# ---------------------------------------------------------------------------
# Host-side sharding / unsharding
# ---------------------------------------------------------------------------

def make_inputs(span_rep, Wl, bl, Wr, br, Wout, bout):
    """Build the per-core input maps (packed layouts, see _build_program)."""
    span_rep = np.ascontiguousarray(np.asarray(span_rep, dtype=np.float32))
    Wl = np.ascontiguousarray(np.asarray(Wl, dtype=np.float32))
    Wr = np.ascontiguousarray(np.asarray(Wr, dtype=np.float32))
    Wout = np.asarray(Wout, dtype=np.float32)
    bl = np.asarray(bl, dtype=np.float32)
    br = np.asarray(br, dtype=np.float32)
    bout = np.asarray(bout, dtype=np.float32)

    # span columns in the flat (r_off) space: span_rep[j, k]^T per j-slot
    span_cols = np.zeros((HID, RCOLS + 80), dtype=np.float32)  # +tail pad
    for grp in GROUPS:
        W = grp["W"]
        for jj, j in enumerate(grp["js"]):
            if j is None:
                continue
            w = 64 - j
            c0 = grp["r_off"] + jj * W
            span_cols[:, c0:c0 + w] = span_rep[j, j + 1:65, :].T

    # packed per-chunk, h-tile-major span: [128, HT * rcols per chunk]
    spanp = np.zeros((128, SPANP_COLS), dtype=np.float16)
    for ch in RCHUNKS:
        for ti in range(HT):
            blk = span_cols[ti * 128:(ti + 1) * 128,
                            ch["rbase"]:ch["rbase"] + ch["rcols"]]
            spanp[:, ch["off4"] + ti * ch["rcols"]:
                  ch["off4"] + (ti + 1) * ch["rcols"]] = blk

    def pack_ht(M, width):  # [512, width] -> [128, HT*width], h-tile-major
        out = np.empty((128, HT * width), dtype=np.float16)
        for ti in range(HT):
            out[:, ti * width:(ti + 1) * width] = M[ti * 128:(ti + 1) * 128, :]
        return out

    # weights packed as [128, 2*HT*HID]: per h_out block `to`, Wl's four
    # h_in 128-blocks then Wr's four
    wp = np.empty((128, 2 * HT * HID), dtype=np.float16)
    for to in range(HT):
        for kind, M in ((0, Wl), (1, Wr)):
            for ti in range(HT):
                c0 = to * 2 * HID + kind * HID + ti * 128
                wp[:, c0:c0 + 128] = \
                    M[ti * 128:(ti + 1) * 128, to * 128:(to + 1) * 128]

    # span_sel: per core. Column q = a*8 + jj of group g holds
    # span_rep[i, j]^T with i = a*8 + core (if that is a valid pair).
    selps = []
    for core in range(NCORES):
        sel = np.zeros((HID, QCOLS), dtype=np.float32)
        for grp in GROUPS:
            n = grp["n"]
            for jj, j in enumerate(grp["js"]):
                if j is None:
                    continue
                for a in range(n):
                    i = a * 8 + core
                    if i < j:
                        sel[:, grp["q_off"] + a * 8 + jj] = span_rep[i, j, :]
        selps.append(pack_ht(sel, QCOLS))

    misc = np.zeros((128, 16), dtype=np.float32)
    misc[:, 0:HT] = (bl + br).reshape(HT, 128).T
    misc[0:OUT, HT] = bout
    misc[:, HT + 1:HT + 1 + OUT * HT] = (
        Wout.reshape(HT, 128, OUT).transpose(1, 0, 2).reshape(128, HT * OUT))

    in_maps = []
    for core in range(NCORES):
        in_maps.append({
            "spanp": spanp,
            "selp": selps[core],
            "wp": wp,
            "misc": misc,
        })
    return in_maps


def scatter_outputs(core_outs):
    """Assemble the full [65, 65, 65, 2] output from per-core [2, SCOLS]."""
    full = np.zeros((N1, N1, N1, OUT), dtype=np.float32)
    for core in range(NCORES):
        oc = core_outs[core]
        for grp in GROUPS:
            W, n = grp["W"], grp["n"]
            for jj, j in enumerate(grp["js"]):
                if j is None:
                    continue
                w = 64 - j
                for a in range(n):
                    i = a * 8 + core
                    if i < j:
                        c0 = grp["s_off"] + (a * 8 + jj) * W
                        full[i, j, j + 1:65, :] = oc[:, c0:c0 + w].T
    return full


def kernel(span_rep, Wl, bl, Wr, br, Wout, bout):
    from concourse.bass_utils import run_bass_kernel_spmd

    nc = _get_compiled()
    in_maps = make_inputs(span_rep, Wl, bl, Wr, br, Wout, bout)
    res = run_bass_kernel_spmd(nc, in_maps, core_ids=list(range(NCORES)))
    core_outs = [res.results[c]["out"] for c in range(NCORES)]
    return scatter_outputs(core_outs)


if __name__ == "__main__":
    rng = np.random.default_rng(0)
    s = 1.0 / np.sqrt(HID)
    inputs = dict(
        span_rep=rng.standard_normal((N1, N1, HID)).astype(np.float32),
        Wl=(rng.standard_normal((HID, HID)) * s).astype(np.float32),
        bl=np.zeros(HID, np.float32),
        Wr=(rng.standard_normal((HID, HID)) * s).astype(np.float32),
        br=np.zeros(HID, np.float32),
        Wout=(rng.standard_normal((HID, OUT)) * s).astype(np.float32),
        bout=np.zeros(OUT, np.float32),
    )
    out = kernel(**inputs)
    print("out", out.shape, out.dtype, np.abs(out).max())
